# revision 2
# baseline (speedup 1.0000x reference)
"""DecoderRNN single-step (embed+ReLU -> GRU cell -> vocab projection -> log_softmax)
as a tensor-parallel Bass/Tile kernel on 8 TRN2 NeuronCores.

Sharding:
  - GRU: hidden dim (1024) split 8x128; core c owns rows [c*128,(c+1)*128) of each
    gate. Gate matvecs run on the TensorEngine with host-pre-transposed weights;
    h_new shards are AllGather'd.
  - Output projection: vocab padded to 53248 = 8*6656, sharded contiguously; each
    core computes its 6656 logits on the TensorEngine (weights pre-transposed and
    laid out on host), plus exp-sums; the scalar exp-sums are AllGather'd so every
    core computes the global logsumexp locally and writes its logp shard.

Contraction layout: the hidden dim is consumed in 8 chunks of 128 with the
permutation h = 8p + j (partition p, chunk j) so that vectors in "column layout"
[128, 8] are plain row-major reshapes of the length-1024 vector, and all weight
tiles are host-side rearrangements with fully-contiguous per-partition DMA rows.

Weights are cast to bf16 on host (inputs/accumulation stay f32); psum accumulation
is f32. Embedding table is replicated; the row gather is an indirect DMA with
per-partition offsets token*128 + p into an [V*128, 8] view of the table.
"""

import numpy as np
import ml_dtypes

import concourse.bass as bass
import concourse.tile as tile
from concourse import bacc, mybir
from concourse.bass_utils import run_bass_kernel_spmd

HIDDEN = 1024
VOCAB = 50257
N_CORES = 8
VPAD = 53248            # 8 * 6656, multiple of 8*128
VSH = VPAD // N_CORES   # 6656 vocab rows per core
NT = VSH // 128         # 52 vocab tiles per core
NG = 4                  # vocab tile groups (DMA granularity)
GT = NT // NG           # 13 tiles per group
GW = VSH // NG          # 1664 columns per group
NJ = HIDDEN // 128      # 8 contraction chunks
PAD_BIAS = -1e30

F32 = mybir.dt.float32
BF16 = mybir.dt.bfloat16
I32 = mybir.dt.int32
BF16_NP = ml_dtypes.bfloat16

_CACHE = {}


def _build_nc():
    nc = bacc.Bacc(
        "TRN2",
        target_bir_lowering=False,
        debug=False,
        num_devices=N_CORES,
    )
    # ---- kernel I/O ----
    emb_d = nc.dram_tensor("emb_v", [VOCAB * 128, 8], F32, kind="ExternalInput")
    token_d = nc.dram_tensor("token32", [1, 1], I32, kind="ExternalInput")
    iota_d = nc.dram_tensor("iota128", [128, 1], F32, kind="ExternalInput")
    hidcol_d = nc.dram_tensor("hid_col", [128, 8], F32, kind="ExternalInput")
    hslice_d = nc.dram_tensor("h_slice", [128, 1], F32, kind="ExternalInput")
    gbias_d = nc.dram_tensor("gru_bias", [128, 4], F32, kind="ExternalInput")
    wih_d = nc.dram_tensor("w_ih_p", [3, 128, NJ, 128], BF16, kind="ExternalInput")
    whh_d = nc.dram_tensor("w_hh_p", [3, 128, NJ, 128], BF16, kind="ExternalInput")
    wout_d = nc.dram_tensor("w_out_p", [NJ, 128, VSH], BF16, kind="ExternalInput")
    bout_d = nc.dram_tensor("b_out_col", [128, NT], F32, kind="ExternalInput")
    logp_d = nc.dram_tensor("logp", [128, NT], F32, kind="ExternalOutput")
    hnew_d = nc.dram_tensor("h_new", [128, 1], F32, kind="ExternalOutput")

    AF = mybir.ActivationFunctionType
    OP = mybir.AluOpType

    with tile.TileContext(nc) as tc:
        with (
            tc.tile_pool(name="small", bufs=1) as small,
            tc.tile_pool(name="wpool", bufs=1) as wpool,
            tc.tile_pool(name="psump", bufs=1, space="PSUM") as psump,
            tc.tile_pool(name="dram", bufs=1, space="DRAM") as dram,
        ):
            # ---- small input loads ----
            tok_i = small.tile([128, 1], I32)
            nc.gpsimd.dma_start(out=tok_i[:], in_=token_d.ap().to_broadcast([128, 1]))
            iota_sb = small.tile([128, 1], F32)
            nc.sync.dma_start(out=iota_sb[:], in_=iota_d.ap())
            hidcol_f = small.tile([128, 8], F32)
            nc.sync.dma_start(out=hidcol_f[:], in_=hidcol_d.ap())
            hsl = small.tile([128, 1], F32)
            nc.sync.dma_start(out=hsl[:], in_=hslice_d.ap())
            gb = small.tile([128, 4], F32)
            nc.sync.dma_start(out=gb[:], in_=gbias_d.ap())
            bout_sb = small.tile([128, NT], F32)
            nc.sync.dma_start(out=bout_sb[:], in_=bout_d.ap())
            ones_sb = small.tile([128, 1], F32)
            nc.vector.memset(ones_sb[:], 1.0)

            # ---- embedding row gather (x = emb[token] in column layout) ----
            tok_f = small.tile([128, 1], F32)
            nc.vector.tensor_copy(tok_f[:], tok_i[:])
            offs_f = small.tile([128, 1], F32)
            nc.vector.tensor_scalar(
                offs_f[:], tok_f[:], 128.0, iota_sb[:], OP.mult, OP.add
            )
            offs_i = small.tile([128, 1], I32)
            nc.vector.tensor_copy(offs_i[:], offs_f[:])
            x_raw = small.tile([128, 8], F32)
            nc.gpsimd.indirect_dma_start(
                out=x_raw[:],
                out_offset=None,
                in_=emb_d.ap(),
                in_offset=bass.IndirectOffsetOnAxis(ap=offs_i[:], axis=0),
            )
            x_rel = small.tile([128, 8], F32)
            nc.vector.tensor_scalar_max(x_rel[:], x_raw[:], 0.0)
            x_bf = small.tile([128, 8], BF16)
            nc.vector.tensor_copy(x_bf[:], x_rel[:])
            h_bf = small.tile([128, 8], BF16)
            nc.vector.tensor_copy(h_bf[:], hidcol_f[:])

            # ---- GRU weight loads ----
            wih_sb = []
            whh_sb = []
            for g in range(3):
                t_ih = small.tile([128, NJ, 128], BF16, tag=f"wih{g}")
                nc.sync.dma_start(out=t_ih[:], in_=wih_d.ap()[g])
                wih_sb.append(t_ih)
                t_hh = small.tile([128, NJ, 128], BF16, tag=f"whh{g}")
                nc.sync.dma_start(out=t_hh[:], in_=whh_d.ap()[g])
                whh_sb.append(t_hh)

            # ---- output projection weight streams ----
            w_sb = [[None] * NJ for _ in range(NG)]
            for g in range(NG):
                for j in range(NJ):
                    t = wpool.tile([128, GW], BF16, tag=f"w{g}_{j}")
                    nc.sync.dma_start(
                        out=t[:], in_=wout_d.ap()[j][:, g * GW : (g + 1) * GW]
                    )
                    w_sb[g][j] = t

            # ---- GRU gate matvecs on PE ----
            # separate psum tiles (one bank each) for r, z, i_n, h_n
            ps_r = psump.tile([128, 1], F32, tag="ps_r")
            ps_z = psump.tile([128, 1], F32, tag="ps_z")
            ps_in = psump.tile([128, 1], F32, tag="ps_in")
            ps_hn = psump.tile([128, 1], F32, tag="ps_hn")
            # r and z accumulate both W_ih and W_hh contributions
            for ps, blocks in (
                (ps_r, ((wih_sb[0], x_bf), (whh_sb[0], h_bf))),
                (ps_z, ((wih_sb[1], x_bf), (whh_sb[1], h_bf))),
                (ps_in, ((wih_sb[2], x_bf),)),
                (ps_hn, ((whh_sb[2], h_bf),)),
            ):
                n_mm = len(blocks) * NJ
                k = 0
                for w, rhs in blocks:
                    for j in range(NJ):
                        nc.tensor.matmul(
                            out=ps[:],
                            lhsT=w[:, j, :],
                            rhs=rhs[:, j : j + 1],
                            start=(k == 0),
                            stop=(k == n_mm - 1),
                        )
                        k += 1

            # ---- GRU elementwise ----
            r_sb = small.tile([128, 1], F32)
            nc.scalar.activation(r_sb[:], ps_r[:], AF.Sigmoid, bias=gb[:, 0:1])
            z_sb = small.tile([128, 1], F32)
            nc.scalar.activation(z_sb[:], ps_z[:], AF.Sigmoid, bias=gb[:, 1:2])
            inb = small.tile([128, 1], F32)
            nc.scalar.activation(inb[:], ps_in[:], AF.Identity, bias=gb[:, 2:3])
            hnb = small.tile([128, 1], F32)
            nc.scalar.activation(hnb[:], ps_hn[:], AF.Identity, bias=gb[:, 3:4])
            rhn = small.tile([128, 1], F32)
            nc.vector.tensor_tensor(rhn[:], r_sb[:], hnb[:], op=OP.mult)
            n_sb = small.tile([128, 1], F32)
            nc.scalar.activation(n_sb[:], rhn[:], AF.Tanh, bias=inb[:, 0:1])
            d_sb = small.tile([128, 1], F32)
            nc.vector.tensor_tensor(d_sb[:], hsl[:], n_sb[:], op=OP.subtract)
            zd = small.tile([128, 1], F32)
            nc.vector.tensor_tensor(zd[:], z_sb[:], d_sb[:], op=OP.mult)
            hnew_sb = small.tile([128, 1], F32)
            nc.vector.tensor_tensor(hnew_sb[:], n_sb[:], zd[:], op=OP.add)

            # preload the exp/ln ACT table set early (hides the ~2.7us load)
            dum0 = small.tile([1, 1], F32)
            nc.vector.memset(dum0[:], 1.0)
            dum1 = small.tile([1, 1], F32)
            nc.scalar.activation(dum1[:], dum0[:], AF.Exp)
            dum2 = small.tile([1, 1], F32)
            nc.scalar.activation(dum2[:], dum1[:], AF.Ln)

            # ---- AllGather h_new shards -> full h_new ----
            cc1_in = dram.tile([128, 1], F32)
            cc1_out = dram.tile([HIDDEN, 1], F32, addr_space="Shared")
            nc.sync.dma_start(out=cc1_in[:], in_=hnew_sb[:])
            nc.gpsimd.collective_compute(
                "AllGather",
                OP.bypass,
                replica_groups=[list(range(N_CORES))],
                ins=[cc1_in[:].opt()],
                outs=[cc1_out[:].opt()],
            )
            nc.sync.dma_start(out=hnew_d.ap(), in_=hnew_sb[:])

            hcol_f = small.tile([128, 8], F32)
            nc.sync.dma_start(
                out=hcol_f[:],
                in_=cc1_out[:].rearrange("(p j) o -> p (j o)", p=128),
            )
            hcol_bf = small.tile([128, 8], BF16)
            nc.vector.tensor_copy(hcol_bf[:], hcol_f[:])

            # ---- output projection: logits + exp-sums, group by group ----
            logits_sb = small.tile([128, NT], F32)
            sums = small.tile([128, NG], F32)
            for g in range(NG):
                ps = psump.tile([128, GT], F32, tag="lps", bufs=2)
                for t in range(GT):
                    for j in range(NJ):
                        nc.tensor.matmul(
                            out=ps[:, t : t + 1],
                            lhsT=w_sb[g][j][:, t * 128 : (t + 1) * 128],
                            rhs=hcol_bf[:, j : j + 1],
                            start=(j == 0),
                            stop=(j == NJ - 1),
                        )
                gsl = slice(g * GT, (g + 1) * GT)
                nc.vector.tensor_tensor(
                    logits_sb[:, gsl], ps[:], bout_sb[:, gsl], op=OP.add
                )
                esc = small.tile([128, GT], F32, tag="esc", bufs=2)
                nc.scalar.activation(
                    esc[:], logits_sb[:, gsl], AF.Exp, accum_out=sums[:, g : g + 1]
                )

            # ---- global logsumexp via AllGather of per-core exp sums ----
            stot = small.tile([128, 1], F32)
            nc.vector.tensor_reduce(stot[:], sums[:], axis=mybir.AxisListType.X, op=OP.add)
            ps_s = psump.tile([1, 1], F32, tag="ps_s")
            nc.tensor.matmul(out=ps_s[:], lhsT=ones_sb[:], rhs=stot[:], start=True, stop=True)
            s_pad = small.tile([1, 8], F32)
            nc.vector.memset(s_pad[:], 0.0)
            nc.scalar.copy(s_pad[0:1, 0:1], ps_s[:])
            cc2_in = dram.tile([1, 8], F32)
            cc2_out = dram.tile([N_CORES, 8], F32, addr_space="Shared")
            nc.sync.dma_start(out=cc2_in[:], in_=s_pad[:])
            nc.gpsimd.collective_compute(
                "AllGather",
                OP.bypass,
                replica_groups=[list(range(N_CORES))],
                ins=[cc2_in[:].opt()],
                outs=[cc2_out[:].opt()],
            )
            sall = small.tile([128, N_CORES], F32)
            nc.gpsimd.dma_start(
                out=sall[:],
                in_=bass.AP(
                    tensor=cc2_out.tensor,
                    offset=cc2_out.offset,
                    ap=[[0, 128], [8, N_CORES]],
                ),
            )
            stot2 = small.tile([128, 1], F32)
            nc.vector.tensor_reduce(
                stot2[:], sall[:], axis=mybir.AxisListType.X, op=OP.add
            )
            lse = small.tile([128, 1], F32)
            nc.scalar.activation(lse[:], stot2[:], AF.Ln)

            logp_sb = small.tile([128, NT], F32)
            nc.vector.tensor_scalar(
                logp_sb[:], logits_sb[:], lse[:], None, OP.subtract
            )
            nc.sync.dma_start(out=logp_d.ap(), in_=logp_sb[:])

    nc.compile()
    return nc


def get_nc():
    if "nc" not in _CACHE:
        _CACHE["nc"] = _build_nc()
    return _CACHE["nc"]


def prepare_in_maps(token, hidden, emb, w_ih, w_hh, b_ih, b_hh, w_out, b_out):
    token = np.asarray(token).reshape(-1)
    hidden = np.asarray(hidden, dtype=np.float32).reshape(HIDDEN)
    emb = np.asarray(emb, dtype=np.float32)
    w_ih = np.asarray(w_ih, dtype=np.float32)
    w_hh = np.asarray(w_hh, dtype=np.float32)
    b_ih = np.asarray(b_ih, dtype=np.float32)
    b_hh = np.asarray(b_hh, dtype=np.float32)
    w_out = np.asarray(w_out, dtype=np.float32)
    b_out = np.asarray(b_out, dtype=np.float32)

    emb_v = np.ascontiguousarray(emb).reshape(VOCAB * 128, 8)
    token32 = np.array([[int(token[0])]], dtype=np.int32)
    iota128 = np.arange(128, dtype=np.float32).reshape(128, 1)
    hid_col = np.ascontiguousarray(hidden.reshape(128, 8))

    # GRU weights: [3h, h] -> per core [3, p, j, m] with h-col = 8p+j, row = g*1024+c*128+m
    def prep_gru(w):
        a = w.reshape(3, N_CORES, 128, 128, NJ)  # (g, c, m, p, j)
        return [
            np.ascontiguousarray(a[:, c].transpose(0, 2, 3, 1)).astype(BF16_NP)
            for c in range(N_CORES)
        ]

    wih_p = prep_gru(w_ih)
    whh_p = prep_gru(w_hh)

    b_r = (b_ih[0:HIDDEN] + b_hh[0:HIDDEN]).reshape(N_CORES, 128)
    b_z = (b_ih[HIDDEN : 2 * HIDDEN] + b_hh[HIDDEN : 2 * HIDDEN]).reshape(N_CORES, 128)
    b_in = b_ih[2 * HIDDEN :].reshape(N_CORES, 128)
    b_hn = b_hh[2 * HIDDEN :].reshape(N_CORES, 128)

    # output projection: pad vocab, [V, h] -> per core [j, p, v] (h-col = 8p+j)
    w_out_pad = np.zeros((VPAD, HIDDEN), dtype=np.float32)
    w_out_pad[:VOCAB] = w_out
    wp = w_out_pad.reshape(N_CORES, VSH, 128, NJ)  # (c, v, p, j)
    b_out_pad = np.full(VPAD, PAD_BIAS, dtype=np.float32)
    b_out_pad[:VOCAB] = b_out
    bp = b_out_pad.reshape(N_CORES, NT, 128)

    in_maps = []
    for c in range(N_CORES):
        in_maps.append(
            {
                "emb_v": emb_v,
                "token32": token32,
                "iota128": iota128,
                "hid_col": hid_col,
                "h_slice": np.ascontiguousarray(
                    hidden[c * 128 : (c + 1) * 128].reshape(128, 1)
                ),
                "gru_bias": np.ascontiguousarray(
                    np.stack([b_r[c], b_z[c], b_in[c], b_hn[c]], axis=1)
                ),
                "w_ih_p": wih_p[c],
                "w_hh_p": whh_p[c],
                "w_out_p": np.ascontiguousarray(wp[c].transpose(2, 1, 0)).astype(
                    BF16_NP
                ),
                "b_out_col": np.ascontiguousarray(bp[c].T),
            }
        )
    return in_maps


def assemble_outputs(results):
    logp_pad = np.empty(VPAD, dtype=np.float32)
    hnew = np.empty(HIDDEN, dtype=np.float32)
    for c in range(N_CORES):
        lp = results[c]["logp"]  # [128, NT], v_local = t*128 + p
        logp_pad[c * VSH : (c + 1) * VSH] = lp.T.reshape(VSH)
        hnew[c * 128 : (c + 1) * 128] = results[c]["h_new"][:, 0]
    return logp_pad[:VOCAB].reshape(1, VOCAB), hnew.reshape(1, 1, HIDDEN)


def run(inputs, **spmd_kwargs):
    nc = get_nc()
    in_maps = prepare_in_maps(**inputs)
    res = run_bass_kernel_spmd(nc, in_maps, core_ids=list(range(N_CORES)), **spmd_kwargs)
    return assemble_outputs(res.results), res


def kernel(**inputs):
    outputs, _ = run(inputs)
    return outputs


# revision 8
# speedup vs baseline: 1.1406x; 1.1406x over previous
"""DecoderRNN single-step (embed+ReLU -> GRU cell -> vocab projection -> log_softmax)
as a tensor-parallel Bass/Tile kernel on 8 TRN2 NeuronCores.

Sharding:
  - GRU: hidden dim (1024) split 8x128; core c owns rows [c*128,(c+1)*128) of each
    gate. Gate matvecs run on the TensorEngine with host-pre-transposed weights;
    h_new shards are AllGather'd.
  - Output projection: vocab padded to 53248 = 8*6656, sharded contiguously; each
    core computes its 6656 logits on the TensorEngine (weights pre-transposed and
    laid out on host), plus exp-sums; the scalar exp-sums are AllGather'd so every
    core computes the global logsumexp locally and writes its logp shard.

Contraction layout: the hidden dim is consumed in 8 chunks of 128 with the
permutation h = 8p + j (partition p, chunk j) so that vectors in "column layout"
[128, 8] are plain row-major reshapes of the length-1024 vector, and all weight
tiles are host-side rearrangements with fully-contiguous per-partition DMA rows.

Weights are cast to bf16 on host (inputs/accumulation stay f32); psum accumulation
is f32. Embedding table is replicated; the row gather is an indirect DMA with
per-partition offsets token*128 + p into an [V*128, 8] view of the table.
"""

import numpy as np
import ml_dtypes

import concourse.bass as bass
import concourse.tile as tile
from concourse import bacc, mybir
from concourse.bass_utils import run_bass_kernel_spmd

HIDDEN = 1024
VOCAB = 50257
N_CORES = 8
VPAD = 53248            # 8 * 6656, multiple of 8*128
VSH = VPAD // N_CORES   # 6656 vocab rows per core
NT = VSH // 128         # 52 vocab tiles per core
NG = 4                  # vocab tile groups (DMA granularity)
GT = NT // NG           # 13 tiles per group
GW = VSH // NG          # 1664 columns per group
NJ = HIDDEN // 128      # 8 contraction chunks
PAD_BIAS = -1e30

F32 = mybir.dt.float32
BF16 = mybir.dt.bfloat16
I32 = mybir.dt.int32
BF16_NP = ml_dtypes.bfloat16

_CACHE = {}


def _build_nc():
    nc = bacc.Bacc(
        "TRN2",
        target_bir_lowering=False,
        debug=False,
        num_devices=N_CORES,
    )
    # ---- kernel I/O ----
    emb_d = nc.dram_tensor("emb_v", [VOCAB * 128, 8], F32, kind="ExternalInput")
    token_d = nc.dram_tensor("token32", [1, 1], I32, kind="ExternalInput")
    iota_d = nc.dram_tensor("iota128", [128, 1], F32, kind="ExternalInput")
    hidcol_d = nc.dram_tensor("hid_col", [128, 8], F32, kind="ExternalInput")
    hslice_d = nc.dram_tensor("h_slice", [128, 1], F32, kind="ExternalInput")
    gbias_d = nc.dram_tensor("gru_bias", [128, 4], F32, kind="ExternalInput")
    wih_d = nc.dram_tensor("w_ih_p", [3, 128, NJ, 128], BF16, kind="ExternalInput")
    whh_d = nc.dram_tensor("w_hh_p", [3, 128, NJ, 128], BF16, kind="ExternalInput")
    wout_d = nc.dram_tensor("w_out_p", [NJ, 128, VSH], BF16, kind="ExternalInput")
    bout_d = nc.dram_tensor("b_out_col", [128, NT], F32, kind="ExternalInput")
    logp_d = nc.dram_tensor("logp", [128, NT], F32, kind="ExternalOutput")
    hnew_d = nc.dram_tensor("h_new", [128, 1], F32, kind="ExternalOutput")

    AF = mybir.ActivationFunctionType
    OP = mybir.AluOpType

    with tile.TileContext(nc) as tc:
        with (
            tc.tile_pool(name="small", bufs=1) as small,
            tc.tile_pool(name="wpool", bufs=1) as wpool,
            tc.tile_pool(name="psump", bufs=1, space="PSUM") as psump,
            tc.tile_pool(name="dram", bufs=1, space="DRAM") as dram,
        ):
            # ---- small input loads ----
            tok_i = small.tile([128, 1], I32)
            nc.gpsimd.dma_start(out=tok_i[:], in_=token_d.ap().to_broadcast([128, 1]))
            iota_sb = small.tile([128, 1], F32)
            nc.sync.dma_start(out=iota_sb[:], in_=iota_d.ap())
            hidcol_f = small.tile([128, 8], F32)
            nc.sync.dma_start(out=hidcol_f[:], in_=hidcol_d.ap())
            hsl = small.tile([128, 1], F32)
            nc.sync.dma_start(out=hsl[:], in_=hslice_d.ap())
            gb = small.tile([128, 4], F32)
            nc.sync.dma_start(out=gb[:], in_=gbias_d.ap())
            bout_sb = small.tile([128, NT], F32)
            nc.sync.dma_start(out=bout_sb[:], in_=bout_d.ap())
            ones_sb = small.tile([128, 1], F32)
            nc.vector.memset(ones_sb[:], 1.0)
            ones_row = small.tile([1, 128], F32)
            nc.vector.memset(ones_row[:], 1.0)
            zero8 = small.tile([1, 8], F32)
            nc.vector.memset(zero8[:], 0.0)

            # first ACT op is an Exp so walrus loads exp_and_others (which also
            # serves every Tanh/Identity/Copy below) once, early
            dum0 = small.tile([1, 1], F32)
            nc.vector.memset(dum0[:], 1.0)
            dum1 = small.tile([1, 1], F32)
            nc.scalar.activation(dum1[:], dum0[:], mybir.ActivationFunctionType.Exp)

            # ---- embedding row gather (x = emb[token] in column layout) ----
            tok_f = small.tile([128, 1], F32)
            nc.vector.tensor_copy(tok_f[:], tok_i[:])
            offs_f = small.tile([128, 1], F32)
            nc.vector.tensor_scalar(
                offs_f[:], tok_f[:], 128.0, iota_sb[:], OP.mult, OP.add
            )
            offs_i = small.tile([128, 1], I32)
            nc.vector.tensor_copy(offs_i[:], offs_f[:])
            x_raw = small.tile([128, 8], F32)
            nc.gpsimd.indirect_dma_start(
                out=x_raw[:],
                out_offset=None,
                in_=emb_d.ap(),
                in_offset=bass.IndirectOffsetOnAxis(ap=offs_i[:], axis=0),
            )
            x_rel = small.tile([128, 8], F32)
            nc.vector.tensor_scalar_max(x_rel[:], x_raw[:], 0.0)
            x_bf = small.tile([128, 8], BF16)
            nc.vector.tensor_copy(x_bf[:], x_rel[:])
            h_bf = small.tile([128, 8], BF16)
            nc.vector.tensor_copy(h_bf[:], hidcol_f[:])

            # ---- GRU weight loads ----
            wih_sb = []
            whh_sb = []
            for g in range(3):
                t_ih = small.tile([128, NJ, 128], BF16, tag=f"wih{g}")
                nc.sync.dma_start(out=t_ih[:], in_=wih_d.ap()[g])
                wih_sb.append(t_ih)
                t_hh = small.tile([128, NJ, 128], BF16, tag=f"whh{g}")
                nc.sync.dma_start(out=t_hh[:], in_=whh_d.ap()[g])
                whh_sb.append(t_hh)

            # ---- output projection weight streams ----
            w_sb = [[None] * NJ for _ in range(NG)]
            for g in range(NG):
                for j in range(NJ):
                    t = wpool.tile([128, GW], BF16, tag=f"w{g}_{j}")
                    nc.sync.dma_start(
                        out=t[:], in_=wout_d.ap()[j][:, g * GW : (g + 1) * GW]
                    )
                    w_sb[g][j] = t

            # ---- GRU gate matvecs on PE ----
            # separate psum tiles (one bank each) for r, z, i_n, h_n
            ps_r = psump.tile([128, 1], F32, tag="ps_r")
            ps_z = psump.tile([128, 1], F32, tag="ps_z")
            ps_in = psump.tile([128, 1], F32, tag="ps_in")
            ps_hn = psump.tile([128, 1], F32, tag="ps_hn")
            # r and z accumulate both W_ih and W_hh contributions
            for ps, blocks in (
                (ps_r, ((wih_sb[0], x_bf), (whh_sb[0], h_bf))),
                (ps_z, ((wih_sb[1], x_bf), (whh_sb[1], h_bf))),
                (ps_in, ((wih_sb[2], x_bf),)),
                (ps_hn, ((whh_sb[2], h_bf),)),
            ):
                n_mm = len(blocks) * NJ
                k = 0
                for w, rhs in blocks:
                    for j in range(NJ):
                        nc.tensor.matmul(
                            out=ps[:],
                            lhsT=w[:, j, :],
                            rhs=rhs[:, j : j + 1],
                            start=(k == 0),
                            stop=(k == n_mm - 1),
                        )
                        k += 1

            # ---- GRU elementwise ----
            # sigmoid(v) = 0.5*tanh(0.5*v) + 0.5 keeps everything in the
            # exp_and_others ACT table set (no sigmoid-set load).
            # gru_bias cols 0/1 hold 0.5*(b_ih+b_hh) for r/z (host-prepped).
            r_t = small.tile([128, 1], F32)
            nc.scalar.activation(r_t[:], ps_r[:], AF.Tanh, bias=gb[:, 0:1], scale=0.5)
            r_sb = small.tile([128, 1], F32)
            nc.vector.tensor_scalar(r_sb[:], r_t[:], 0.5, 0.5, OP.mult, OP.add)
            z_t = small.tile([128, 1], F32)
            nc.scalar.activation(z_t[:], ps_z[:], AF.Tanh, bias=gb[:, 1:2], scale=0.5)
            z_sb = small.tile([128, 1], F32)
            nc.vector.tensor_scalar(z_sb[:], z_t[:], 0.5, 0.5, OP.mult, OP.add)
            inb = small.tile([128, 1], F32)
            nc.scalar.activation(inb[:], ps_in[:], AF.Identity, bias=gb[:, 2:3])
            hnb = small.tile([128, 1], F32)
            nc.scalar.activation(hnb[:], ps_hn[:], AF.Identity, bias=gb[:, 3:4])
            rhn = small.tile([128, 1], F32)
            nc.vector.tensor_tensor(rhn[:], r_sb[:], hnb[:], op=OP.mult)
            n_sb = small.tile([128, 1], F32)
            nc.scalar.activation(n_sb[:], rhn[:], AF.Tanh, bias=inb[:, 0:1])
            d_sb = small.tile([128, 1], F32)
            nc.vector.tensor_tensor(d_sb[:], hsl[:], n_sb[:], op=OP.subtract)
            zd = small.tile([128, 1], F32)
            nc.vector.tensor_tensor(zd[:], z_sb[:], d_sb[:], op=OP.mult)
            hnew_sb = small.tile([128, 1], F32)
            nc.vector.tensor_tensor(hnew_sb[:], n_sb[:], zd[:], op=OP.add)

            # ---- AllGather h_new shards -> full h_new ----
            # NOTE: these small mid-kernel DMAs go on the ACT HWDGE ring
            # (nc.scalar) so they don't queue behind the w_out weight stream
            # on the Sync ring.
            cc1_in = dram.tile([128, 1], F32)
            cc1_out = dram.tile([HIDDEN, 1], F32, addr_space="Shared")
            nc.scalar.dma_start(out=cc1_in[:], in_=hnew_sb[:])
            nc.gpsimd.collective_compute(
                "AllGather",
                OP.bypass,
                replica_groups=[list(range(N_CORES))],
                ins=[cc1_in[:].opt()],
                outs=[cc1_out[:].opt()],
            )
            nc.scalar.dma_start(out=hnew_d.ap(), in_=hnew_sb[:])

            hcol_f = small.tile([128, 8], F32)
            nc.scalar.dma_start(
                out=hcol_f[:],
                in_=cc1_out[:].rearrange("(p j) o -> p (j o)", p=128),
            )
            hcol_bf = small.tile([128, 8], BF16)
            nc.vector.tensor_copy(hcol_bf[:], hcol_f[:])

            # ---- output projection: logits + exp-sums, group by group ----
            logits_sb = small.tile([128, NT], F32)
            sums = small.tile([128, NG], F32)
            for g in range(NG):
                ps = psump.tile([128, GT], F32, tag="lps", bufs=2)
                for t in range(GT):
                    for j in range(NJ):
                        nc.tensor.matmul(
                            out=ps[:, t : t + 1],
                            lhsT=w_sb[g][j][:, t * 128 : (t + 1) * 128],
                            rhs=hcol_bf[:, j : j + 1],
                            start=(j == 0),
                            stop=(j == NJ - 1),
                        )
                gsl = slice(g * GT, (g + 1) * GT)
                nc.vector.tensor_tensor(
                    logits_sb[:, gsl], ps[:], bout_sb[:, gsl], op=OP.add
                )
                esc = small.tile([128, GT], F32, tag="esc", bufs=2)
                nc.scalar.activation(
                    esc[:], logits_sb[:, gsl], AF.Exp, accum_out=sums[:, g : g + 1]
                )

            # ---- global logsumexp via AllGather of per-core exp sums ----
            stot = small.tile([128, 1], F32)
            nc.vector.tensor_reduce(stot[:], sums[:], axis=mybir.AxisListType.X, op=OP.add)
            # partition-reduce via PE: ps_s[0,0] = sum_p stot[p]
            ps_s = psump.tile([1, 1], F32, tag="ps_s")
            nc.tensor.matmul(out=ps_s[:], lhsT=ones_sb[:], rhs=stot[:], start=True, stop=True)
            s_sb = small.tile([1, 1], F32)
            nc.scalar.copy(s_sb[:], ps_s[:])
            # replicate the scalar across 8 lanes so both collective bounce
            # buffers are single-descriptor contiguous reads/writes; the 8x
            # over-count is folded into the Ln scale below.
            s8 = small.tile([1, 8], F32)
            nc.vector.tensor_scalar(s8[:], zero8[:], s_sb[0:1, 0:1], None, OP.add)
            cc2_in = dram.tile([1, 8], F32)
            cc2_out = dram.tile([N_CORES, 8], F32, addr_space="Shared")
            nc.scalar.dma_start(out=cc2_in[:], in_=s8[:])
            # load the natural_log table set while AG2 is in flight
            duml = small.tile([1, 1], F32)
            nc.scalar.activation(duml[:], dum1[:], AF.Ln)
            nc.gpsimd.collective_compute(
                "AllGather",
                OP.bypass,
                replica_groups=[list(range(N_CORES))],
                ins=[cc2_in[:].opt()],
                outs=[cc2_out[:].opt()],
            )
            s64 = small.tile([1, 64], F32)
            nc.scalar.dma_start(
                out=s64[:], in_=cc2_out[:].rearrange("(o a) b -> o (a b)", o=1)
            )
            stot2 = small.tile([1, 1], F32)
            nc.vector.tensor_reduce(
                stot2[:], s64[:], axis=mybir.AxisListType.X, op=OP.add
            )
            # lse = ln(sum_c s_c) = ln(0.125 * sum(s64))
            lse1 = small.tile([1, 1], F32)
            nc.scalar.activation(lse1[:], stot2[:], AF.Ln, scale=0.125)
            # broadcast lse to all partitions via PE (ones_row.T @ lse1)
            ps_l = psump.tile([128, 1], F32, tag="ps_l")
            nc.tensor.matmul(out=ps_l[:], lhsT=ones_row[:], rhs=lse1[:], start=True, stop=True)
            lse_b = small.tile([128, 1], F32)
            nc.scalar.copy(lse_b[:], ps_l[:])

            logp_sb = small.tile([128, NT], F32)
            nc.vector.tensor_scalar(
                logp_sb[:], logits_sb[:], lse_b[:], None, OP.subtract
            )
            nc.scalar.dma_start(out=logp_d.ap(), in_=logp_sb[:])

    nc.compile()
    return nc


def get_nc():
    if "nc" not in _CACHE:
        _CACHE["nc"] = _build_nc()
    return _CACHE["nc"]


def prepare_in_maps(token, hidden, emb, w_ih, w_hh, b_ih, b_hh, w_out, b_out):
    token = np.asarray(token).reshape(-1)
    hidden = np.asarray(hidden, dtype=np.float32).reshape(HIDDEN)
    emb = np.asarray(emb, dtype=np.float32)
    w_ih = np.asarray(w_ih, dtype=np.float32)
    w_hh = np.asarray(w_hh, dtype=np.float32)
    b_ih = np.asarray(b_ih, dtype=np.float32)
    b_hh = np.asarray(b_hh, dtype=np.float32)
    w_out = np.asarray(w_out, dtype=np.float32)
    b_out = np.asarray(b_out, dtype=np.float32)

    emb_v = np.ascontiguousarray(emb).reshape(VOCAB * 128, 8)
    token32 = np.array([[int(token[0])]], dtype=np.int32)
    iota128 = np.arange(128, dtype=np.float32).reshape(128, 1)
    hid_col = np.ascontiguousarray(hidden.reshape(128, 8))

    # GRU weights: [3h, h] -> per core [3, p, j, m] with h-col = 8p+j, row = g*1024+c*128+m
    def prep_gru(w):
        a = w.reshape(3, N_CORES, 128, 128, NJ)  # (g, c, m, p, j)
        return [
            np.ascontiguousarray(a[:, c].transpose(0, 2, 3, 1)).astype(BF16_NP)
            for c in range(N_CORES)
        ]

    wih_p = prep_gru(w_ih)
    whh_p = prep_gru(w_hh)

    # r/z biases pre-scaled by 0.5 for the tanh-based sigmoid
    b_r = (0.5 * (b_ih[0:HIDDEN] + b_hh[0:HIDDEN])).reshape(N_CORES, 128)
    b_z = (
        0.5 * (b_ih[HIDDEN : 2 * HIDDEN] + b_hh[HIDDEN : 2 * HIDDEN])
    ).reshape(N_CORES, 128)
    b_in = b_ih[2 * HIDDEN :].reshape(N_CORES, 128)
    b_hn = b_hh[2 * HIDDEN :].reshape(N_CORES, 128)

    # output projection: pad vocab, [V, h] -> per core [j, p, v] (h-col = 8p+j)
    w_out_pad = np.zeros((VPAD, HIDDEN), dtype=np.float32)
    w_out_pad[:VOCAB] = w_out
    wp = w_out_pad.reshape(N_CORES, VSH, 128, NJ)  # (c, v, p, j)
    b_out_pad = np.full(VPAD, PAD_BIAS, dtype=np.float32)
    b_out_pad[:VOCAB] = b_out
    bp = b_out_pad.reshape(N_CORES, NT, 128)

    in_maps = []
    for c in range(N_CORES):
        in_maps.append(
            {
                "emb_v": emb_v,
                "token32": token32,
                "iota128": iota128,
                "hid_col": hid_col,
                "h_slice": np.ascontiguousarray(
                    hidden[c * 128 : (c + 1) * 128].reshape(128, 1)
                ),
                "gru_bias": np.ascontiguousarray(
                    np.stack([b_r[c], b_z[c], b_in[c], b_hn[c]], axis=1)
                ),
                "w_ih_p": wih_p[c],
                "w_hh_p": whh_p[c],
                "w_out_p": np.ascontiguousarray(wp[c].transpose(2, 1, 0)).astype(
                    BF16_NP
                ),
                "b_out_col": np.ascontiguousarray(bp[c].T),
            }
        )
    return in_maps


def assemble_outputs(results):
    logp_pad = np.empty(VPAD, dtype=np.float32)
    hnew = np.empty(HIDDEN, dtype=np.float32)
    for c in range(N_CORES):
        lp = results[c]["logp"]  # [128, NT], v_local = t*128 + p
        logp_pad[c * VSH : (c + 1) * VSH] = lp.T.reshape(VSH)
        hnew[c * 128 : (c + 1) * 128] = results[c]["h_new"][:, 0]
    return logp_pad[:VOCAB].reshape(1, VOCAB), hnew.reshape(1, 1, HIDDEN)


def run(inputs, **spmd_kwargs):
    nc = get_nc()
    in_maps = prepare_in_maps(**inputs)
    res = run_bass_kernel_spmd(nc, in_maps, core_ids=list(range(N_CORES)), **spmd_kwargs)
    return assemble_outputs(res.results), res


def kernel(**inputs):
    outputs, _ = run(inputs)
    return outputs


# revision 11
# speedup vs baseline: 1.2586x; 1.1034x over previous
"""DecoderRNN single-step (embed+ReLU -> GRU cell -> vocab projection -> log_softmax)
as a tensor-parallel Bass/Tile kernel on 8 TRN2 NeuronCores.

Sharding:
  - GRU: hidden dim (1024) split 8x128; core c owns rows [c*128,(c+1)*128) of each
    gate. Gate matvecs run on the TensorEngine with host-pre-transposed weights;
    h_new shards are AllGather'd.
  - Output projection: vocab padded to 53248 = 8*6656, sharded contiguously; each
    core computes its 6656 logits on the TensorEngine (weights pre-transposed and
    laid out on host), plus exp-sums; the scalar exp-sums are AllGather'd so every
    core computes the global logsumexp locally and writes its logp shard.

Contraction layout: the hidden dim is consumed in 8 chunks of 128 with the
permutation h = 8p + j (partition p, chunk j) so that vectors in "column layout"
[128, 8] are plain row-major reshapes of the length-1024 vector, and all weight
tiles are host-side rearrangements with fully-contiguous per-partition DMA rows.

Weights are cast to bf16 on host (inputs/accumulation stay f32); psum accumulation
is f32. Embedding table is replicated; the row gather is an indirect DMA with
per-partition offsets token*128 + p into an [V*128, 8] view of the table.
"""

import numpy as np
import ml_dtypes

import concourse.bass as bass
import concourse.tile as tile
from concourse import bacc, mybir
from concourse.bass_utils import run_bass_kernel_spmd

HIDDEN = 1024
VOCAB = 50257
N_CORES = 8
VPAD = 53248            # 8 * 6656, multiple of 8*128
VSH = VPAD // N_CORES   # 6656 vocab rows per core
NT = VSH // 128         # 52 vocab tiles per core
NG = 2                  # psum/evacuation groups
GT = NT // NG           # 26 tiles per group
NJ = HIDDEN // 128      # 8 contraction chunks
PAD_BIAS = -1e30

F32 = mybir.dt.float32
BF16 = mybir.dt.bfloat16
I32 = mybir.dt.int32
BF16_NP = ml_dtypes.bfloat16

_CACHE = {}


def _build_nc():
    nc = bacc.Bacc(
        "TRN2",
        target_bir_lowering=False,
        debug=False,
        num_devices=N_CORES,
    )
    # ---- kernel I/O ----
    emb_d = nc.dram_tensor("emb_v", [VOCAB * 128, 8], F32, kind="ExternalInput")
    token_d = nc.dram_tensor("token32", [1, 1], I32, kind="ExternalInput")
    iota_d = nc.dram_tensor("iota128", [128, 1], F32, kind="ExternalInput")
    hidcol_d = nc.dram_tensor("hid_col", [128, 8], F32, kind="ExternalInput")
    hslice_d = nc.dram_tensor("h_slice", [128, 1], F32, kind="ExternalInput")
    gbias_d = nc.dram_tensor("gru_bias", [128, 4], F32, kind="ExternalInput")
    wih_d = nc.dram_tensor("w_ih_p", [3, 128, NJ, 128], BF16, kind="ExternalInput")
    whh_d = nc.dram_tensor("w_hh_p", [3, 128, NJ, 128], BF16, kind="ExternalInput")
    wout_d = nc.dram_tensor("w_out_p", [NJ, 128, VSH], BF16, kind="ExternalInput")
    bout_d = nc.dram_tensor("b_out_col", [128, NT], F32, kind="ExternalInput")
    logp_d = nc.dram_tensor("logp", [128, NT], F32, kind="ExternalOutput")
    hnew_d = nc.dram_tensor("h_new", [128, 1], F32, kind="ExternalOutput")

    AF = mybir.ActivationFunctionType
    OP = mybir.AluOpType

    with tile.TileContext(nc) as tc:
        with (
            tc.tile_pool(name="small", bufs=1) as small,
            tc.tile_pool(name="wpool", bufs=1) as wpool,
            tc.tile_pool(name="psump", bufs=1, space="PSUM") as psump,
            tc.tile_pool(name="dram", bufs=1, space="DRAM") as dram,
        ):
            # ---- small input loads ----
            tok_i = small.tile([128, 1], I32)
            nc.gpsimd.dma_start(out=tok_i[:], in_=token_d.ap().to_broadcast([128, 1]))
            iota_sb = small.tile([128, 1], F32)
            nc.sync.dma_start(out=iota_sb[:], in_=iota_d.ap())
            hidcol_f = small.tile([128, 8], F32)
            nc.sync.dma_start(out=hidcol_f[:], in_=hidcol_d.ap())
            hsl = small.tile([128, 1], F32)
            nc.sync.dma_start(out=hsl[:], in_=hslice_d.ap())
            gb = small.tile([128, 4], F32)
            nc.sync.dma_start(out=gb[:], in_=gbias_d.ap())
            bout_sb = small.tile([128, NT], F32)
            nc.sync.dma_start(out=bout_sb[:], in_=bout_d.ap())
            ones_sb = small.tile([128, 1], F32)
            nc.vector.memset(ones_sb[:], 1.0)
            ones_row = small.tile([1, 128], F32)
            nc.vector.memset(ones_row[:], 1.0)
            zero8 = small.tile([1, 8], F32)
            nc.vector.memset(zero8[:], 0.0)

            # first ACT op is an Exp so walrus loads exp_and_others (which also
            # serves every Tanh/Identity/Copy below) once, early
            dum0 = small.tile([1, 1], F32)
            nc.vector.memset(dum0[:], 1.0)
            dum1 = small.tile([1, 1], F32)
            nc.scalar.activation(dum1[:], dum0[:], mybir.ActivationFunctionType.Exp)

            # ---- embedding row gather (x = emb[token] in column layout) ----
            tok_f = small.tile([128, 1], F32)
            nc.vector.tensor_copy(tok_f[:], tok_i[:])
            offs_f = small.tile([128, 1], F32)
            nc.vector.tensor_scalar(
                offs_f[:], tok_f[:], 128.0, iota_sb[:], OP.mult, OP.add
            )
            offs_i = small.tile([128, 1], I32)
            nc.vector.tensor_copy(offs_i[:], offs_f[:])
            x_raw = small.tile([128, 8], F32)
            nc.gpsimd.indirect_dma_start(
                out=x_raw[:],
                out_offset=None,
                in_=emb_d.ap(),
                in_offset=bass.IndirectOffsetOnAxis(ap=offs_i[:], axis=0),
            )
            x_rel = small.tile([128, 8], F32)
            nc.vector.tensor_scalar_max(x_rel[:], x_raw[:], 0.0)
            x_bf = small.tile([128, 8], BF16)
            nc.vector.tensor_copy(x_bf[:], x_rel[:])
            h_bf = small.tile([128, 8], BF16)
            nc.vector.tensor_copy(h_bf[:], hidcol_f[:])

            # ---- GRU weight loads ----
            wih_sb = []
            whh_sb = []
            for g in range(3):
                t_ih = small.tile([128, NJ, 128], BF16, tag=f"wih{g}")
                nc.sync.dma_start(out=t_ih[:], in_=wih_d.ap()[g])
                wih_sb.append(t_ih)
                t_hh = small.tile([128, NJ, 128], BF16, tag=f"whh{g}")
                nc.sync.dma_start(out=t_hh[:], in_=whh_d.ap()[g])
                whh_sb.append(t_hh)

            # ---- output projection weight streams ----
            # one full-width DMA per contraction chunk: 8 DMAs x 1.7MB fill all
            # 8 HWDGE completion lanes once, no lane-FIFO round-trip stalls
            w_sb = []
            for j in range(NJ):
                t = wpool.tile([128, VSH], BF16, tag=f"w{j}")
                nc.sync.dma_start(out=t[:], in_=wout_d.ap()[j])
                w_sb.append(t)

            # ---- GRU gate matvecs on PE ----
            # separate psum tiles (one bank each) for r, z, i_n, h_n
            ps_r = psump.tile([128, 1], F32, tag="ps_r")
            ps_z = psump.tile([128, 1], F32, tag="ps_z")
            ps_in = psump.tile([128, 1], F32, tag="ps_in")
            ps_hn = psump.tile([128, 1], F32, tag="ps_hn")
            # r and z accumulate both W_ih and W_hh contributions
            for ps, blocks in (
                (ps_r, ((wih_sb[0], x_bf), (whh_sb[0], h_bf))),
                (ps_z, ((wih_sb[1], x_bf), (whh_sb[1], h_bf))),
                (ps_in, ((wih_sb[2], x_bf),)),
                (ps_hn, ((whh_sb[2], h_bf),)),
            ):
                n_mm = len(blocks) * NJ
                k = 0
                for w, rhs in blocks:
                    for j in range(NJ):
                        nc.tensor.matmul(
                            out=ps[:],
                            lhsT=w[:, j, :],
                            rhs=rhs[:, j : j + 1],
                            start=(k == 0),
                            stop=(k == n_mm - 1),
                        )
                        k += 1

            # ---- GRU elementwise ----
            # sigmoid(v) = 0.5*tanh(0.5*v) + 0.5 keeps everything in the
            # exp_and_others ACT table set (no sigmoid-set load).
            # gru_bias cols 0/1 hold 0.5*(b_ih+b_hh) for r/z (host-prepped).
            r_t = small.tile([128, 1], F32)
            nc.scalar.activation(r_t[:], ps_r[:], AF.Tanh, bias=gb[:, 0:1], scale=0.5)
            r_sb = small.tile([128, 1], F32)
            nc.vector.tensor_scalar(r_sb[:], r_t[:], 0.5, 0.5, OP.mult, OP.add)
            z_t = small.tile([128, 1], F32)
            nc.scalar.activation(z_t[:], ps_z[:], AF.Tanh, bias=gb[:, 1:2], scale=0.5)
            z_sb = small.tile([128, 1], F32)
            nc.vector.tensor_scalar(z_sb[:], z_t[:], 0.5, 0.5, OP.mult, OP.add)
            inb = small.tile([128, 1], F32)
            nc.scalar.activation(inb[:], ps_in[:], AF.Identity, bias=gb[:, 2:3])
            hnb = small.tile([128, 1], F32)
            nc.scalar.activation(hnb[:], ps_hn[:], AF.Identity, bias=gb[:, 3:4])
            rhn = small.tile([128, 1], F32)
            nc.vector.tensor_tensor(rhn[:], r_sb[:], hnb[:], op=OP.mult)
            n_sb = small.tile([128, 1], F32)
            nc.scalar.activation(n_sb[:], rhn[:], AF.Tanh, bias=inb[:, 0:1])
            d_sb = small.tile([128, 1], F32)
            nc.vector.tensor_tensor(d_sb[:], hsl[:], n_sb[:], op=OP.subtract)
            zd = small.tile([128, 1], F32)
            nc.vector.tensor_tensor(zd[:], z_sb[:], d_sb[:], op=OP.mult)
            hnew_sb = small.tile([128, 1], F32)
            nc.vector.tensor_tensor(hnew_sb[:], n_sb[:], zd[:], op=OP.add)

            # ---- AllGather h_new shards -> full h_new ----
            # NOTE: these small mid-kernel DMAs go on the ACT HWDGE ring
            # (nc.scalar) so they don't queue behind the w_out weight stream
            # on the Sync ring.
            cc1_in = dram.tile([128, 1], F32)
            cc1_out = dram.tile([HIDDEN, 1], F32, addr_space="Shared")
            nc.scalar.dma_start(out=cc1_in[:], in_=hnew_sb[:])
            nc.gpsimd.collective_compute(
                "AllGather",
                OP.bypass,
                replica_groups=[list(range(N_CORES))],
                ins=[cc1_in[:].opt()],
                outs=[cc1_out[:].opt()],
            )
            nc.scalar.dma_start(out=hnew_d.ap(), in_=hnew_sb[:])

            hcol_f = small.tile([128, 8], F32)
            nc.scalar.dma_start(
                out=hcol_f[:],
                in_=cc1_out[:].rearrange("(p j) o -> p (j o)", p=128),
            )
            hcol_bf = small.tile([128, 8], BF16)
            nc.vector.tensor_copy(hcol_bf[:], hcol_f[:])

            # ---- output projection: logits + exp-sums, group by group ----
            logits_sb = small.tile([128, NT], F32)
            sums = small.tile([128, NG], F32)
            for g in range(NG):
                ps = psump.tile([128, GT], F32, tag="lps", bufs=2)
                for t in range(GT):
                    tg = g * GT + t
                    for j in range(NJ):
                        nc.tensor.matmul(
                            out=ps[:, t : t + 1],
                            lhsT=w_sb[j][:, tg * 128 : (tg + 1) * 128],
                            rhs=hcol_bf[:, j : j + 1],
                            start=(j == 0),
                            stop=(j == NJ - 1),
                        )
                gsl = slice(g * GT, (g + 1) * GT)
                nc.vector.tensor_tensor(
                    logits_sb[:, gsl], ps[:], bout_sb[:, gsl], op=OP.add
                )
                esc = small.tile([128, GT], F32, tag="esc", bufs=2)
                nc.scalar.activation(
                    esc[:], logits_sb[:, gsl], AF.Exp, accum_out=sums[:, g : g + 1]
                )

            # ---- global logsumexp via AllGather of per-core exp sums ----
            stot = small.tile([128, 1], F32)
            nc.vector.tensor_reduce(stot[:], sums[:], axis=mybir.AxisListType.X, op=OP.add)
            # partition-reduce via PE: ps_s[0,0] = sum_p stot[p]
            ps_s = psump.tile([1, 1], F32, tag="ps_s")
            nc.tensor.matmul(out=ps_s[:], lhsT=ones_sb[:], rhs=stot[:], start=True, stop=True)
            s_sb = small.tile([1, 1], F32)
            nc.scalar.copy(s_sb[:], ps_s[:])
            # replicate the scalar across 8 lanes so both collective bounce
            # buffers are single-descriptor contiguous reads/writes; the 8x
            # over-count is folded into the Ln scale below.
            s8 = small.tile([1, 8], F32)
            nc.vector.tensor_scalar(s8[:], zero8[:], s_sb[0:1, 0:1], None, OP.add)
            cc2_in = dram.tile([1, 8], F32)
            cc2_out = dram.tile([N_CORES, 8], F32, addr_space="Shared")
            nc.scalar.dma_start(out=cc2_in[:], in_=s8[:])
            # load the natural_log table set while AG2 is in flight
            duml = small.tile([1, 1], F32)
            nc.scalar.activation(duml[:], dum1[:], AF.Ln)
            nc.gpsimd.collective_compute(
                "AllGather",
                OP.bypass,
                replica_groups=[list(range(N_CORES))],
                ins=[cc2_in[:].opt()],
                outs=[cc2_out[:].opt()],
            )
            s64 = small.tile([1, 64], F32)
            nc.scalar.dma_start(
                out=s64[:], in_=cc2_out[:].rearrange("(o a) b -> o (a b)", o=1)
            )
            stot2 = small.tile([1, 1], F32)
            nc.vector.tensor_reduce(
                stot2[:], s64[:], axis=mybir.AxisListType.X, op=OP.add
            )
            # lse = ln(sum_c s_c) = ln(0.125 * sum(s64))
            lse1 = small.tile([1, 1], F32)
            nc.scalar.activation(lse1[:], stot2[:], AF.Ln, scale=0.125)
            # broadcast lse to all partitions via PE (ones_row.T @ lse1)
            ps_l = psump.tile([128, 1], F32, tag="ps_l")
            nc.tensor.matmul(out=ps_l[:], lhsT=ones_row[:], rhs=lse1[:], start=True, stop=True)
            lse_b = small.tile([128, 1], F32)
            nc.scalar.copy(lse_b[:], ps_l[:])

            logp_sb = small.tile([128, NT], F32)
            nc.vector.tensor_scalar(
                logp_sb[:], logits_sb[:], lse_b[:], None, OP.subtract
            )
            nc.scalar.dma_start(out=logp_d.ap(), in_=logp_sb[:])

    nc.compile()
    return nc


def get_nc():
    if "nc" not in _CACHE:
        _CACHE["nc"] = _build_nc()
    return _CACHE["nc"]


def prepare_in_maps(token, hidden, emb, w_ih, w_hh, b_ih, b_hh, w_out, b_out):
    token = np.asarray(token).reshape(-1)
    hidden = np.asarray(hidden, dtype=np.float32).reshape(HIDDEN)
    emb = np.asarray(emb, dtype=np.float32)
    w_ih = np.asarray(w_ih, dtype=np.float32)
    w_hh = np.asarray(w_hh, dtype=np.float32)
    b_ih = np.asarray(b_ih, dtype=np.float32)
    b_hh = np.asarray(b_hh, dtype=np.float32)
    w_out = np.asarray(w_out, dtype=np.float32)
    b_out = np.asarray(b_out, dtype=np.float32)

    emb_v = np.ascontiguousarray(emb).reshape(VOCAB * 128, 8)
    token32 = np.array([[int(token[0])]], dtype=np.int32)
    iota128 = np.arange(128, dtype=np.float32).reshape(128, 1)
    hid_col = np.ascontiguousarray(hidden.reshape(128, 8))

    # GRU weights: [3h, h] -> per core [3, p, j, m] with h-col = 8p+j, row = g*1024+c*128+m
    def prep_gru(w):
        a = w.reshape(3, N_CORES, 128, 128, NJ)  # (g, c, m, p, j)
        return [
            np.ascontiguousarray(a[:, c].transpose(0, 2, 3, 1)).astype(BF16_NP)
            for c in range(N_CORES)
        ]

    wih_p = prep_gru(w_ih)
    whh_p = prep_gru(w_hh)

    # r/z biases pre-scaled by 0.5 for the tanh-based sigmoid
    b_r = (0.5 * (b_ih[0:HIDDEN] + b_hh[0:HIDDEN])).reshape(N_CORES, 128)
    b_z = (
        0.5 * (b_ih[HIDDEN : 2 * HIDDEN] + b_hh[HIDDEN : 2 * HIDDEN])
    ).reshape(N_CORES, 128)
    b_in = b_ih[2 * HIDDEN :].reshape(N_CORES, 128)
    b_hn = b_hh[2 * HIDDEN :].reshape(N_CORES, 128)

    # output projection: pad vocab, [V, h] -> per core [j, p, v] (h-col = 8p+j)
    w_out_pad = np.zeros((VPAD, HIDDEN), dtype=np.float32)
    w_out_pad[:VOCAB] = w_out
    wp = w_out_pad.reshape(N_CORES, VSH, 128, NJ)  # (c, v, p, j)
    b_out_pad = np.full(VPAD, PAD_BIAS, dtype=np.float32)
    b_out_pad[:VOCAB] = b_out
    bp = b_out_pad.reshape(N_CORES, NT, 128)

    in_maps = []
    for c in range(N_CORES):
        in_maps.append(
            {
                "emb_v": emb_v,
                "token32": token32,
                "iota128": iota128,
                "hid_col": hid_col,
                "h_slice": np.ascontiguousarray(
                    hidden[c * 128 : (c + 1) * 128].reshape(128, 1)
                ),
                "gru_bias": np.ascontiguousarray(
                    np.stack([b_r[c], b_z[c], b_in[c], b_hn[c]], axis=1)
                ),
                "w_ih_p": wih_p[c],
                "w_hh_p": whh_p[c],
                "w_out_p": np.ascontiguousarray(wp[c].transpose(2, 1, 0)).astype(
                    BF16_NP
                ),
                "b_out_col": np.ascontiguousarray(bp[c].T),
            }
        )
    return in_maps


def assemble_outputs(results):
    logp_pad = np.empty(VPAD, dtype=np.float32)
    hnew = np.empty(HIDDEN, dtype=np.float32)
    for c in range(N_CORES):
        lp = results[c]["logp"]  # [128, NT], v_local = t*128 + p
        logp_pad[c * VSH : (c + 1) * VSH] = lp.T.reshape(VSH)
        hnew[c * 128 : (c + 1) * 128] = results[c]["h_new"][:, 0]
    return logp_pad[:VOCAB].reshape(1, VOCAB), hnew.reshape(1, 1, HIDDEN)


def run(inputs, **spmd_kwargs):
    nc = get_nc()
    in_maps = prepare_in_maps(**inputs)
    res = run_bass_kernel_spmd(nc, in_maps, core_ids=list(range(N_CORES)), **spmd_kwargs)
    return assemble_outputs(res.results), res


def kernel(**inputs):
    outputs, _ = run(inputs)
    return outputs
